# revision 2
# baseline (speedup 1.0000x reference)
"""Bass/Trainium2 kernel for nn_HardNegativeContrastiveLoss.

Split of work:
  - Host one-time (cached): the reference's fixed-key Gumbel matrices
    (jax.random.key(42), generated on the CPU backend) and a per-row
    top-256 index table of g_neg (input-independent).
  - Host per call (~0.1s): exact replication of the reference's mining
    (masked argmax for the positive, masked top-8 for negatives) via the
    top-256 table + per-class g_pos blocks; L2-normalize features in f32
    and round to bf16.
  - Device (8 NeuronCores, data-parallel over batch): each core receives
    its 1024-row bf16 shard + gather indices packed in one tensor
    (~1.2MB/core on the wire instead of 16MB), AllGathers the full
    normalized matrix over NeuronLink, dma_gathers the positive + 8
    negative candidate rows per row, computes dot products on VectorE
    (bf16 in, f32 accum), top-3 hard negatives via the DVE max op, and
    the per-row logsumexp loss. Host sums the 8192 per-row losses.
  - The compiled program and the jitted SPMD executor are built once and
    cached; steady-state calls only ship ~9.6MB over the axon tunnel.
"""

import numpy as np
from types import SimpleNamespace

B = 8192
D = 512
NCORES = 8
RPC = B // NCORES  # rows per core
P = 128
NTILE = RPC // P  # 8 row-tiles per core
M = 8  # NUM_NEG_CANDIDATES
Q = 1 + M  # gathered rows per target row: positive + 8 negative candidates
KTOP = 256  # precomputed per-row top-K of g_neg (covers max class size + M)
IDXC = (Q * P) // 16  # wrapped idx columns per tile = 72
TEMPERATURE = 0.5

_CACHE = {}


def _bf16():
    import ml_dtypes

    return np.dtype(ml_dtypes.bfloat16)


def _gumbels():
    """Generate the reference's fixed-key Gumbel matrices on the CPU
    backend (bit-identical threefry bits; log/exp may differ from other
    backends by ULPs, which cannot meaningfully change argmax/top-k of
    continuous random values)."""
    import jax
    import jax.numpy as jnp

    dev = jax.devices("cpu")[0]
    with jax.default_device(dev):
        kp, kn = jax.random.split(jax.random.key(42))
        g_pos = np.asarray(jax.random.gumbel(kp, (B, B), dtype=jnp.float32))
        g_neg = np.asarray(jax.random.gumbel(kn, (B, B), dtype=jnp.float32))
    return g_pos, g_neg


def _init_mining():
    if "g_pos" in _CACHE:
        return
    g_pos, g_neg = _gumbels()
    # Input-independent per-row top-KTOP of g_neg, descending, ties by
    # lower index (the jax.lax.top_k rule).
    part = np.argpartition(-g_neg, KTOP - 1, axis=1)[:, :KTOP]
    part.sort(axis=1)
    vals = np.take_along_axis(g_neg, part, axis=1)
    sel = np.argsort(-vals, axis=1, kind="stable")
    _CACHE["negtop"] = np.take_along_axis(part, sel, axis=1).astype(np.int32)
    _CACHE["g_pos"] = g_pos


def _mine_slow(labels):
    """Exact replication of the reference mining for arbitrary labels
    (only used when a class is so large the top-KTOP table can't cover
    it)."""
    import jax
    import jax.numpy as jnp

    g_pos = _CACHE["g_pos"]
    dev = jax.devices("cpu")[0]
    with jax.default_device(dev):
        _, kn = jax.random.split(jax.random.key(42))
        g_neg = np.asarray(jax.random.gumbel(kn, (B, B), dtype=jnp.float32))
    labels = np.asarray(labels).reshape(-1)
    same = labels[:, None] == labels[None, :]
    neg_inf = np.float32(-np.inf)
    pos_mask = same.copy()
    np.fill_diagonal(pos_mask, False)
    gp = np.where(pos_mask, g_pos, neg_inf)
    pos_j = gp.argmax(axis=1)
    gn = np.where(~same, g_neg, neg_inf)
    KP = 64
    part = np.argpartition(-gn, KP - 1, axis=1)[:, :KP]
    part.sort(axis=1)
    v0 = np.take_along_axis(gn, part, axis=1)
    sel = np.argsort(-v0, axis=1, kind="stable")[:, :M]
    neg_idx = np.take_along_axis(part, sel, axis=1)
    return pos_j, neg_idx


def _mine(labels):
    """Replicates reference mining exactly. Returns pos_j [B], neg_idx [B, M]."""
    _init_mining()
    labels = np.asarray(labels).reshape(-1)
    counts = np.bincount(labels, minlength=int(labels.max()) + 1)
    if counts.max() + M > KTOP:
        return _mine_slow(labels)
    g_pos = _CACHE["g_pos"]
    negtop = _CACHE["negtop"]
    # Negatives: first M different-label entries of the per-row
    # descending top-KTOP — identical to top-8 of the masked row because
    # at most counts.max() same-label entries can precede them.
    lab_t = labels[negtop]
    invalid = lab_t == labels[:, None]
    sel = np.argsort(invalid, axis=1, kind="stable")[:, :M]
    sel.sort(axis=1)  # restore descending-value order among the selected
    neg_idx = np.take_along_axis(negtop, sel, axis=1)
    # Positive: per-class masked argmax (only same-label columns are
    # candidates). First-max tie-break preserved: block columns are in
    # ascending global index order.
    pos_j = np.zeros(B, np.int64)
    order = np.argsort(labels, kind="stable")
    sl = labels[order]
    bounds = np.searchsorted(sl, np.arange(int(sl[-1]) + 2))
    for c in range(len(bounds) - 1):
        rows = order[bounds[c]:bounds[c + 1]]
        n = len(rows)
        if n == 0:
            continue
        if n == 1:
            pos_j[rows[0]] = 0  # all-(-inf) row: argmax is 0
            continue
        block = g_pos[np.ix_(rows, rows)]
        np.fill_diagonal(block, -np.inf)
        pos_j[rows] = rows[np.argmax(block, axis=1)]
    return pos_j, neg_idx


def _wrap_idx(arr):
    """arr: [..., N] index list -> wrapped int16 layout [..., 128, N//16]
    (dma_gather idxs: unwrapped[i] = idxs[i % 16, i // 16], replicated
    across the eight 16-partition blocks)."""
    n = arr.shape[-1]
    s = np.arange(n // 16)
    p = np.arange(P)
    m = s[None, :] * 16 + (p[:, None] % 16)  # [128, n//16]
    return arr[..., m].astype(np.int16)


def _build_program(variant="e"):
    """variant "e": bf16 on the wire, f32 on the device (convert before
    an f32 AllGather; f32 gathers). variant "b": bf16 end-to-end
    (bf16 AllGather + bf16 gathers)."""
    import concourse.tile as tile
    from concourse import mybir
    from contextlib import ExitStack

    f32 = mybir.dt.float32
    bf16 = mybir.dt.bfloat16
    i16 = mybir.dt.int16
    Act = mybir.ActivationFunctionType
    X = mybir.AxisListType.X

    import concourse.bacc as bacc

    nc = bacc.Bacc("TRN2", target_bir_lowering=False, debug=False, num_devices=NCORES)
    # Packed per-core input: cols 0:D = this core's bf16 normalized rows,
    # cols D:D+8 = wrapped positive idx, D+8:D+IDXC = wrapped negative
    # idxs (int16 bits) for the row-tile.
    blob = nc.declare_dram_parameter("blob", [RPC, D + IDXC], bf16, isOutput=False)
    lossout = nc.declare_dram_parameter("loss", [NTILE, P], f32, isOutput=True)

    gdt = f32 if variant == "e" else bf16

    with ExitStack() as ctx:
        tc = ctx.enter_context(tile.TileContext(nc))
        dram = ctx.enter_context(tc.tile_pool(name="dram", bufs=1, space="DRAM"))
        big = ctx.enter_context(tc.tile_pool(name="big", bufs=3))
        mid = ctx.enter_context(tc.tile_pool(name="mid", bufs=3))
        scr = ctx.enter_context(tc.tile_pool(name="scr", bufs=2))
        sml = ctx.enter_context(tc.tile_pool(name="sml", bufs=4))

        agin = dram.tile([RPC, D], gdt)
        fall = dram.tile([B, D], gdt)
        if variant == "e":
            # upconvert the bf16 shard to f32 in DRAM before the collective
            for g in range(NTILE):
                rows = slice(g * P, (g + 1) * P)
                cb = mid.tile([P, D], bf16, tag="cb")
                nc.gpsimd.dma_start(cb[:], blob[rows, 0:D])
                cf = scr.tile([P, D], f32, tag="cf")
                nc.vector.tensor_copy(cf[:], cb[:])
                nc.gpsimd.dma_start(agin[rows, :], cf[:])
        else:
            nc.gpsimd.dma_start(agin[:], blob[:, 0:D])
        nc.gpsimd.collective_compute(
            "AllGather",
            mybir.AluOpType.bypass,
            replica_groups=[list(range(NCORES))],
            ins=[agin[:].opt()],
            outs=[fall[:].opt()],
        )

        for g in range(NTILE):
            rows = slice(g * P, (g + 1) * P)
            git = sml.tile([P, IDXC], i16, tag="git")
            nc.gpsimd.dma_start(git[:], blob[rows, D:D + IDXC].bitcast(i16))
            xt = mid.tile([P, D], gdt, tag="xt")
            nc.gpsimd.dma_start(xt[:], agin[rows, :])

            pg = mid.tile([P, D], gdt, tag="pg")
            nc.gpsimd.dma_gather(
                pg[:].rearrange("p (q d) -> p q d", q=1),
                fall[:, :],
                git[:, 0:8],
                num_idxs=P,
                num_idxs_reg=P,
                elem_size=D,
            )
            ng = big.tile([P, M * D], gdt, tag="ng")
            nc.gpsimd.dma_gather(
                ng[:].rearrange("p (q d) -> p q d", q=M),
                fall[:, :],
                git[:, 8:IDXC],
                num_idxs=M * P,
                num_idxs_reg=M * P,
                elem_size=D,
            )

            # dots col 1 = positive, cols 2..9 = negatives (sims: inputs
            # are unit-normalized, so no norm correction needed)
            prod = scr.tile([P, Q * D], gdt, tag="prod")
            nc.vector.tensor_mul(prod[:, 0:D], xt[:], pg[:])
            for m in range(M):
                nc.vector.tensor_mul(
                    prod[:, (1 + m) * D:(2 + m) * D], xt[:], ng[:, m * D:(m + 1) * D]
                )
            dots = sml.tile([P, 16], f32, tag="dots")
            nc.vector.reduce_sum(
                dots[:, 1:1 + Q],
                prod[:].rearrange("p (m d) -> p m d", m=Q),
                axis=X,
            )

            # top-3 hard negatives (max op returns top-8 sorted desc)
            top8 = sml.tile([P, 8], f32, tag="top8")
            nc.vector.max(top8[:], dots[:, 2:2 + M])

            # logsumexp over logits = sims/T = 2*sims: cols [pos, h1, h2, h3]
            mx = sml.tile([P, 4], f32, tag="mx")
            nc.vector.tensor_max(mx[:, 0:1], dots[:, 1:2], top8[:, 0:1])
            nm2 = sml.tile([P, 4], f32, tag="nm2")
            nc.vector.tensor_scalar_mul(nm2[:, 0:1], mx[:, 0:1], -2.0)
            lg = sml.tile([P, 4], f32, tag="lg")
            nc.vector.tensor_copy(lg[:, 0:1], dots[:, 1:2])
            nc.vector.tensor_copy(lg[:, 1:4], top8[:, 0:3])
            ex = sml.tile([P, 4], f32, tag="ex")
            nc.scalar.activation(ex[:], lg[:], Act.Exp, bias=nm2[:, 0:1], scale=2.0)
            s4 = sml.tile([P, 4], f32, tag="s4")
            nc.vector.reduce_sum(s4[:, 0:1], ex[:], axis=X)
            lns = sml.tile([P, 4], f32, tag="lns")
            nc.scalar.activation(lns[:, 0:1], s4[:, 0:1], Act.Ln)
            # loss = lns + 2*(mx - pos)
            df = sml.tile([P, 4], f32, tag="df")
            nc.vector.tensor_sub(df[:, 0:1], mx[:, 0:1], dots[:, 1:2])
            lt = sml.tile([P, 4], f32, tag="lt")
            nc.vector.tensor_scalar_mul(lt[:, 0:1], df[:, 0:1], 2.0)
            lo = sml.tile([P, 4], f32, tag="lo")
            nc.vector.tensor_add(lo[:, 0:1], lt[:, 0:1], lns[:, 0:1])
            nc.gpsimd.dma_start(lossout[g, :], lo[:, 0:1])

    nc.compile()
    return nc


def _make_executor(nc):
    """Build the jitted SPMD executor once (run_bass_via_pjrt re-jits on
    every call; this is the same lowering with a cached jit)."""
    import jax
    from jax.sharding import Mesh, PartitionSpec
    from jax.experimental.shard_map import shard_map
    from concourse import bass2jax, mybir

    bass2jax.install_neuronx_cc_hook()
    partition_name = nc.partition_id_tensor.name if nc.partition_id_tensor else None
    in_names, out_names, out_avals, zero_shapes = [], [], [], []
    for alloc in nc.m.functions[0].allocations:
        if not isinstance(alloc, mybir.MemoryLocationSet):
            continue
        if alloc.kind not in ("ExternalInput", "ExternalOutput"):
            continue
        name = alloc.memorylocations[0].name
        if alloc.kind == "ExternalInput":
            if name != partition_name:
                in_names.append(name)
        else:
            out_names.append(name)
            shape = tuple(alloc.tensor_shape)
            dtype = mybir.dt.np(alloc.dtype)
            out_avals.append(jax.core.ShapedArray(shape, dtype))
            zero_shapes.append((shape, dtype))
    n_params = len(in_names)
    n_outs = len(out_avals)
    all_in_names = list(in_names) + list(out_names)
    if partition_name is not None:
        all_in_names.append(partition_name)

    def _body(*args):
        operands = list(args)
        if partition_name is not None:
            operands.append(bass2jax.partition_id_tensor())
        outs = bass2jax._bass_exec_p.bind(
            *operands,
            out_avals=tuple(out_avals),
            in_names=tuple(all_in_names),
            out_names=tuple(out_names),
            lowering_input_output_aliases=(),
            sim_require_finite=True,
            sim_require_nnan=True,
            nc=nc,
        )
        return tuple(outs)

    devices = jax.devices()[:NCORES]
    mesh = Mesh(np.asarray(devices), ("core",))
    in_specs = (PartitionSpec("core"),) * (n_params + n_outs)
    out_specs = (PartitionSpec("core"),) * len(out_names)
    # No donation: the kernel writes every loss element, so the pre-zeroed
    # output-backing buffers are never consumed and can live on device
    # across calls.
    sharded = jax.jit(
        shard_map(_body, mesh=mesh, in_specs=in_specs, out_specs=out_specs,
                  check_rep=False),
        keep_unused=True,
    )
    shard = jax.sharding.NamedSharding(mesh, PartitionSpec("core"))
    dev_zeros = [
        jax.device_put(np.zeros((NCORES * s[0], *s[1:]), dt), shard)
        for s, dt in zero_shapes
    ]
    for z in dev_zeros:
        z.block_until_ready()
    return sharded, dev_zeros


def _pack_inputs(features, pos_j, neg_idx):
    bf16 = _bf16()
    feat = np.asarray(features, dtype=np.float32)
    norms = np.sqrt(np.einsum("ij,ij->i", feat, feat, dtype=np.float32))
    fnorm = feat / np.maximum(norms, np.float32(1e-12))[:, None]
    fb = fnorm.astype(bf16)  # [B, D] bf16

    # wrapped idx blocks per tile: positive [128, 8], negatives [128, 64]
    pj = pos_j.reshape(NCORES, NTILE, P)
    pw = _wrap_idx(pj)  # [C, T, 128, 8]
    nj = neg_idx.reshape(NCORES, NTILE, P, M).transpose(0, 1, 3, 2)
    nw = _wrap_idx(nj.reshape(NCORES, NTILE, M * P))  # [C, T, 128, 64]
    wrapped = np.concatenate([pw, nw], axis=3)  # [C, T, 128, IDXC]

    blob = np.empty((NCORES, RPC, D + IDXC), dtype=bf16)
    blob[:, :, :D] = fb.reshape(NCORES, RPC, D)
    blob[:, :, D:] = wrapped.reshape(NCORES, RPC, IDXC).view(bf16)
    return blob.reshape(NCORES * RPC, D + IDXC)


def _run(features, labels, trace=False):
    import time

    if "nc" not in _CACHE:
        _CACHE["nc"] = _build_program("b")
    if "fn" not in _CACHE:
        _CACHE["fn"] = _make_executor(_CACHE["nc"])
    fn, dev_zeros = _CACHE["fn"]

    pos_j, neg_idx = _mine(labels)
    blob = _pack_inputs(features, pos_j, neg_idx)

    t0 = time.time()
    outs = fn(blob, *dev_zeros)
    losses = np.asarray(outs[0], dtype=np.float64).reshape(-1)
    wall_ns = (time.time() - t0) * 1e9
    out = np.float32(losses.sum() / B)
    res = SimpleNamespace(exec_time_ns=None, results=None)
    return out, res, wall_ns


def kernel(features, labels):
    out, _, _ = _run(features, labels)
    return out


# revision 3
# speedup vs baseline: 1.5520x; 1.5520x over previous
"""Bass/Trainium2 kernel for nn_HardNegativeContrastiveLoss.

Split of work:
  - Host one-time (cached): the reference's fixed-key Gumbel matrices
    (jax.random.key(42), generated on the CPU backend) and a per-row
    top-256 index table of g_neg (input-independent).
  - Host per call (~0.1s): exact replication of the reference's mining
    (masked argmax for the positive, masked top-8 for negatives) via the
    top-256 table + per-class g_pos blocks; L2-normalize features in f32
    and round to bf16.
  - Device (8 NeuronCores, data-parallel over batch): each core receives
    its 1024-row bf16 shard + gather indices packed in one tensor
    (~1.2MB/core on the wire instead of 16MB), AllGathers the full
    normalized matrix over NeuronLink, dma_gathers the positive + 8
    negative candidate rows per row, computes dot products on VectorE
    (bf16 in, f32 accum), top-3 hard negatives via the DVE max op, and
    the per-row logsumexp loss. Host sums the 8192 per-row losses.
  - The compiled program and the jitted SPMD executor are built once and
    cached; steady-state calls only ship ~9.6MB over the axon tunnel.
"""

import numpy as np
from types import SimpleNamespace

B = 8192
D = 512
NCORES = 8
RPC = B // NCORES  # rows per core
P = 128
NTILE = RPC // P  # 8 row-tiles per core
M = 8  # NUM_NEG_CANDIDATES
Q = 1 + M  # gathered rows per target row: positive + 8 negative candidates
KTOP = 256  # precomputed per-row top-K of g_neg (covers max class size + M)
IDXC = (Q * P) // 16  # wrapped idx columns per tile = 72
TEMPERATURE = 0.5
FSCALE = 8.0  # f8 variant: pre-scale so components sit in e4m3 normal range

_CACHE = {}
_VARIANT = "f8"


def _bf16():
    import ml_dtypes

    return np.dtype(ml_dtypes.bfloat16)


def _gumbels():
    """Generate the reference's fixed-key Gumbel matrices on the CPU
    backend (bit-identical threefry bits; log/exp may differ from other
    backends by ULPs, which cannot meaningfully change argmax/top-k of
    continuous random values)."""
    import jax
    import jax.numpy as jnp

    dev = jax.devices("cpu")[0]
    with jax.default_device(dev):
        kp, kn = jax.random.split(jax.random.key(42))
        g_pos = np.asarray(jax.random.gumbel(kp, (B, B), dtype=jnp.float32))
        g_neg = np.asarray(jax.random.gumbel(kn, (B, B), dtype=jnp.float32))
    return g_pos, g_neg


def _init_mining():
    if "g_pos" in _CACHE:
        return
    g_pos, g_neg = _gumbels()
    # Input-independent per-row top-KTOP of g_neg, descending, ties by
    # lower index (the jax.lax.top_k rule).
    part = np.argpartition(-g_neg, KTOP - 1, axis=1)[:, :KTOP]
    part.sort(axis=1)
    vals = np.take_along_axis(g_neg, part, axis=1)
    sel = np.argsort(-vals, axis=1, kind="stable")
    _CACHE["negtop"] = np.take_along_axis(part, sel, axis=1).astype(np.int32)
    _CACHE["g_pos"] = g_pos


def _mine_slow(labels):
    """Exact replication of the reference mining for arbitrary labels
    (only used when a class is so large the top-KTOP table can't cover
    it)."""
    import jax
    import jax.numpy as jnp

    g_pos = _CACHE["g_pos"]
    dev = jax.devices("cpu")[0]
    with jax.default_device(dev):
        _, kn = jax.random.split(jax.random.key(42))
        g_neg = np.asarray(jax.random.gumbel(kn, (B, B), dtype=jnp.float32))
    labels = np.asarray(labels).reshape(-1)
    same = labels[:, None] == labels[None, :]
    neg_inf = np.float32(-np.inf)
    pos_mask = same.copy()
    np.fill_diagonal(pos_mask, False)
    gp = np.where(pos_mask, g_pos, neg_inf)
    pos_j = gp.argmax(axis=1)
    gn = np.where(~same, g_neg, neg_inf)
    KP = 64
    part = np.argpartition(-gn, KP - 1, axis=1)[:, :KP]
    part.sort(axis=1)
    v0 = np.take_along_axis(gn, part, axis=1)
    sel = np.argsort(-v0, axis=1, kind="stable")[:, :M]
    neg_idx = np.take_along_axis(part, sel, axis=1)
    return pos_j, neg_idx


def _mine(labels):
    """Replicates reference mining exactly. Returns pos_j [B], neg_idx [B, M]."""
    _init_mining()
    labels = np.asarray(labels).reshape(-1)
    counts = np.bincount(labels, minlength=int(labels.max()) + 1)
    if counts.max() + M > KTOP:
        return _mine_slow(labels)
    g_pos = _CACHE["g_pos"]
    negtop = _CACHE["negtop"]
    # Negatives: first M different-label entries of the per-row
    # descending top-KTOP — identical to top-8 of the masked row because
    # at most counts.max() same-label entries can precede them.
    lab_t = labels[negtop]
    invalid = lab_t == labels[:, None]
    sel = np.argsort(invalid, axis=1, kind="stable")[:, :M]
    sel.sort(axis=1)  # restore descending-value order among the selected
    neg_idx = np.take_along_axis(negtop, sel, axis=1)
    # Positive: per-class masked argmax (only same-label columns are
    # candidates). First-max tie-break preserved: block columns are in
    # ascending global index order.
    pos_j = np.zeros(B, np.int64)
    order = np.argsort(labels, kind="stable")
    sl = labels[order]
    bounds = np.searchsorted(sl, np.arange(int(sl[-1]) + 2))
    for c in range(len(bounds) - 1):
        rows = order[bounds[c]:bounds[c + 1]]
        n = len(rows)
        if n == 0:
            continue
        if n == 1:
            pos_j[rows[0]] = 0  # all-(-inf) row: argmax is 0
            continue
        block = g_pos[np.ix_(rows, rows)]
        np.fill_diagonal(block, -np.inf)
        pos_j[rows] = rows[np.argmax(block, axis=1)]
    return pos_j, neg_idx


def _wrap_idx(arr):
    """arr: [..., N] index list -> wrapped int16 layout [..., 128, N//16]
    (dma_gather idxs: unwrapped[i] = idxs[i % 16, i // 16], replicated
    across the eight 16-partition blocks)."""
    n = arr.shape[-1]
    s = np.arange(n // 16)
    p = np.arange(P)
    m = s[None, :] * 16 + (p[:, None] % 16)  # [128, n//16]
    return arr[..., m].astype(np.int16)


def _build_program(variant="b"):
    """variant "b": bf16 end-to-end (bf16 AllGather + bf16 gathers).
    variant "e": bf16 on the wire, f32 on the device. variant "f8":
    fp8(e4m3, x8-scaled) on the wire and in HBM, bf16 compute."""
    import concourse.tile as tile
    from concourse import mybir
    from contextlib import ExitStack

    f32 = mybir.dt.float32
    bf16 = mybir.dt.bfloat16
    f8 = mybir.dt.float8e4
    i16 = mybir.dt.int16
    Act = mybir.ActivationFunctionType
    X = mybir.AxisListType.X

    import concourse.bacc as bacc

    nc = bacc.Bacc("TRN2", target_bir_lowering=False, debug=False, num_devices=NCORES)
    # Packed per-core input: cols 0:D = this core's normalized rows,
    # then wrapped positive idx (8 i16) + wrapped negative idxs (64 i16)
    # per row-tile, stored as raw bytes in the blob dtype.
    gdt = {"b": bf16, "e": f32, "f8": f8}[variant]
    wdt = f8 if variant == "f8" else bf16  # wire dtype of the blob
    idxc_w = IDXC * 2 if variant == "f8" else IDXC  # idx cols in blob units
    # logits on device are (SCALE**2)*sims for f8 (features pre-scaled by
    # SCALE to sit in e4m3's normal range); fold 1/SCALE**2 into the
    # logsumexp constants.
    lsc = 2.0 / (FSCALE * FSCALE) if variant == "f8" else 2.0

    blob = nc.declare_dram_parameter("blob", [RPC, D + idxc_w], wdt, isOutput=False)
    lossout = nc.declare_dram_parameter("loss", [NTILE, P], f32, isOutput=True)

    with ExitStack() as ctx:
        tc = ctx.enter_context(tile.TileContext(nc))
        dram = ctx.enter_context(tc.tile_pool(name="dram", bufs=1, space="DRAM"))
        big = ctx.enter_context(tc.tile_pool(name="big", bufs=3))
        mid = ctx.enter_context(tc.tile_pool(name="mid", bufs=3))
        scr = ctx.enter_context(tc.tile_pool(name="scr", bufs=2))
        sml = ctx.enter_context(tc.tile_pool(name="sml", bufs=4))

        agin = dram.tile([RPC, D], gdt)
        fall = dram.tile([B, D], gdt)
        if variant == "e":
            # upconvert the bf16 shard to f32 in DRAM before the collective
            for g in range(NTILE):
                rows = slice(g * P, (g + 1) * P)
                cb = mid.tile([P, D], bf16, tag="cb")
                nc.gpsimd.dma_start(cb[:], blob[rows, 0:D])
                cf = scr.tile([P, D], f32, tag="cf")
                nc.vector.tensor_copy(cf[:], cb[:])
                nc.gpsimd.dma_start(agin[rows, :], cf[:])
        else:
            nc.gpsimd.dma_start(agin[:], blob[:, 0:D])
        nc.gpsimd.collective_compute(
            "AllGather",
            mybir.AluOpType.bypass,
            replica_groups=[list(range(NCORES))],
            ins=[agin[:].opt()],
            outs=[fall[:].opt()],
        )

        for g in range(NTILE):
            rows = slice(g * P, (g + 1) * P)
            git = sml.tile([P, IDXC], i16, tag="git")
            nc.gpsimd.dma_start(git[:], blob[rows, D:D + idxc_w].bitcast(i16))
            xt = mid.tile([P, D], gdt, tag="xt")
            nc.gpsimd.dma_start(xt[:], agin[rows, :])

            pg = mid.tile([P, D], gdt, tag="pg")
            nc.gpsimd.dma_gather(
                pg[:].rearrange("p (q d) -> p q d", q=1),
                fall[:, :],
                git[:, 0:8],
                num_idxs=P,
                num_idxs_reg=P,
                elem_size=D,
            )
            ng = big.tile([P, M * D], gdt, tag="ng")
            nc.gpsimd.dma_gather(
                ng[:].rearrange("p (q d) -> p q d", q=M),
                fall[:, :],
                git[:, 8:IDXC],
                num_idxs=M * P,
                num_idxs_reg=M * P,
                elem_size=D,
            )

            if variant == "f8":
                # upconvert fp8 to bf16 for the DVE multiplies
                xb = mid.tile([P, D], bf16, tag="xb")
                nc.vector.tensor_copy(xb[:], xt[:])
                pb = mid.tile([P, D], bf16, tag="pb")
                nc.vector.tensor_copy(pb[:], pg[:])
                nb = big.tile([P, M * D], bf16, tag="nb")
                nc.vector.tensor_copy(nb[:], ng[:])
                xt, pg, ng = xb, pb, nb
                pdt = bf16
            else:
                pdt = gdt

            # dots col 1 = positive, cols 2..9 = negatives (sims: inputs
            # are unit-normalized, so no norm correction needed)
            prod = scr.tile([P, Q * D], pdt, tag="prod")
            nc.vector.tensor_mul(prod[:, 0:D], xt[:], pg[:])
            for m in range(M):
                nc.vector.tensor_mul(
                    prod[:, (1 + m) * D:(2 + m) * D], xt[:], ng[:, m * D:(m + 1) * D]
                )
            dots = sml.tile([P, 16], f32, tag="dots")
            nc.vector.reduce_sum(
                dots[:, 1:1 + Q],
                prod[:].rearrange("p (m d) -> p m d", m=Q),
                axis=X,
            )

            # top-3 hard negatives (max op returns top-8 sorted desc)
            top8 = sml.tile([P, 8], f32, tag="top8")
            nc.vector.max(top8[:], dots[:, 2:2 + M])

            # logsumexp over logits = sims/T = 2*sims: cols [pos, h1, h2, h3]
            mx = sml.tile([P, 4], f32, tag="mx")
            nc.vector.tensor_max(mx[:, 0:1], dots[:, 1:2], top8[:, 0:1])
            nm2 = sml.tile([P, 4], f32, tag="nm2")
            nc.vector.tensor_scalar_mul(nm2[:, 0:1], mx[:, 0:1], -lsc)
            lg = sml.tile([P, 4], f32, tag="lg")
            nc.vector.tensor_copy(lg[:, 0:1], dots[:, 1:2])
            nc.vector.tensor_copy(lg[:, 1:4], top8[:, 0:3])
            ex = sml.tile([P, 4], f32, tag="ex")
            nc.scalar.activation(ex[:], lg[:], Act.Exp, bias=nm2[:, 0:1], scale=lsc)
            s4 = sml.tile([P, 4], f32, tag="s4")
            nc.vector.reduce_sum(s4[:, 0:1], ex[:], axis=X)
            lns = sml.tile([P, 4], f32, tag="lns")
            nc.scalar.activation(lns[:, 0:1], s4[:, 0:1], Act.Ln)
            # loss = lns + lsc*(mx - pos)
            df = sml.tile([P, 4], f32, tag="df")
            nc.vector.tensor_sub(df[:, 0:1], mx[:, 0:1], dots[:, 1:2])
            lt = sml.tile([P, 4], f32, tag="lt")
            nc.vector.tensor_scalar_mul(lt[:, 0:1], df[:, 0:1], lsc)
            lo = sml.tile([P, 4], f32, tag="lo")
            nc.vector.tensor_add(lo[:, 0:1], lt[:, 0:1], lns[:, 0:1])
            nc.gpsimd.dma_start(lossout[g, :], lo[:, 0:1])

    nc.compile()
    return nc


def _make_executor(nc):
    """Build the jitted SPMD executor once (run_bass_via_pjrt re-jits on
    every call; this is the same lowering with a cached jit)."""
    import jax
    from jax.sharding import Mesh, PartitionSpec
    from jax.experimental.shard_map import shard_map
    from concourse import bass2jax, mybir

    bass2jax.install_neuronx_cc_hook()
    partition_name = nc.partition_id_tensor.name if nc.partition_id_tensor else None
    in_names, out_names, out_avals, zero_shapes = [], [], [], []
    for alloc in nc.m.functions[0].allocations:
        if not isinstance(alloc, mybir.MemoryLocationSet):
            continue
        if alloc.kind not in ("ExternalInput", "ExternalOutput"):
            continue
        name = alloc.memorylocations[0].name
        if alloc.kind == "ExternalInput":
            if name != partition_name:
                in_names.append(name)
        else:
            out_names.append(name)
            shape = tuple(alloc.tensor_shape)
            dtype = mybir.dt.np(alloc.dtype)
            out_avals.append(jax.core.ShapedArray(shape, dtype))
            zero_shapes.append((shape, dtype))
    n_params = len(in_names)
    n_outs = len(out_avals)
    all_in_names = list(in_names) + list(out_names)
    if partition_name is not None:
        all_in_names.append(partition_name)

    def _body(*args):
        operands = list(args)
        if partition_name is not None:
            operands.append(bass2jax.partition_id_tensor())
        outs = bass2jax._bass_exec_p.bind(
            *operands,
            out_avals=tuple(out_avals),
            in_names=tuple(all_in_names),
            out_names=tuple(out_names),
            lowering_input_output_aliases=(),
            sim_require_finite=True,
            sim_require_nnan=True,
            nc=nc,
        )
        return tuple(outs)

    devices = jax.devices()[:NCORES]
    mesh = Mesh(np.asarray(devices), ("core",))
    in_specs = (PartitionSpec("core"),) * (n_params + n_outs)
    out_specs = (PartitionSpec("core"),) * len(out_names)
    # No donation: the kernel writes every loss element, so the pre-zeroed
    # output-backing buffers are never consumed and can live on device
    # across calls.
    sharded = jax.jit(
        shard_map(_body, mesh=mesh, in_specs=in_specs, out_specs=out_specs,
                  check_rep=False),
        keep_unused=True,
    )
    shard = jax.sharding.NamedSharding(mesh, PartitionSpec("core"))
    dev_zeros = [
        jax.device_put(np.zeros((NCORES * s[0], *s[1:]), dt), shard)
        for s, dt in zero_shapes
    ]
    for z in dev_zeros:
        z.block_until_ready()
    return sharded, dev_zeros


def _pack_inputs(features, pos_j, neg_idx, variant="b"):
    import ml_dtypes

    feat = np.asarray(features, dtype=np.float32)
    norms = np.sqrt(np.einsum("ij,ij->i", feat, feat, dtype=np.float32))
    fnorm = feat / np.maximum(norms, np.float32(1e-12))[:, None]
    if variant == "f8":
        wdt = np.dtype(ml_dtypes.float8_e4m3)
        fb = (fnorm * np.float32(FSCALE)).astype(wdt)
    else:
        wdt = np.dtype(ml_dtypes.bfloat16)
        fb = fnorm.astype(wdt)
    idxc_w = (IDXC * 2) // wdt.itemsize  # idx cols in blob dtype units

    # wrapped idx blocks per tile: positive [128, 8], negatives [128, 64]
    pj = pos_j.reshape(NCORES, NTILE, P)
    pw = _wrap_idx(pj)  # [C, T, 128, 8]
    nj = neg_idx.reshape(NCORES, NTILE, P, M).transpose(0, 1, 3, 2)
    nw = _wrap_idx(nj.reshape(NCORES, NTILE, M * P))  # [C, T, 128, 64]
    wrapped = np.concatenate([pw, nw], axis=3)  # [C, T, 128, IDXC]

    blob = np.empty((NCORES, RPC, D + idxc_w), dtype=wdt)
    blob[:, :, :D] = fb.reshape(NCORES, RPC, D)
    blob[:, :, D:] = np.ascontiguousarray(
        wrapped.reshape(NCORES, RPC, IDXC)).view(wdt)
    return blob.reshape(NCORES * RPC, D + idxc_w)


def _run(features, labels, trace=False):
    import time

    variant = _VARIANT
    if "nc" not in _CACHE:
        _CACHE["nc"] = _build_program(variant)
    if "fn" not in _CACHE:
        _CACHE["fn"] = _make_executor(_CACHE["nc"])
    fn, dev_zeros = _CACHE["fn"]

    pos_j, neg_idx = _mine(labels)
    blob = _pack_inputs(features, pos_j, neg_idx, variant)

    t0 = time.time()
    outs = fn(blob, *dev_zeros)
    losses = np.asarray(outs[0], dtype=np.float64).reshape(-1)
    wall_ns = (time.time() - t0) * 1e9
    out = np.float32(losses.sum() / B)
    res = SimpleNamespace(exec_time_ns=None, results=None)
    return out, res, wall_ns


def kernel(features, labels):
    out, _, _ = _run(features, labels)
    return out


# revision 4
# speedup vs baseline: 1.7697x; 1.1403x over previous
"""Bass/Trainium2 kernel for nn_HardNegativeContrastiveLoss.

Split of work:
  - Host one-time (cached): the reference's fixed-key Gumbel matrices
    (jax.random.key(42), generated on the CPU backend) and a per-row
    top-256 index table of g_neg (input-independent).
  - Host per call (~0.1s): exact replication of the reference's mining
    (masked argmax for the positive, masked top-8 for negatives) via the
    top-256 table + per-class g_pos blocks; L2-normalize features in f32
    and round to bf16.
  - Device (8 NeuronCores, data-parallel over batch): each core receives
    its 1024-row bf16 shard + gather indices packed in one tensor
    (~1.2MB/core on the wire instead of 16MB), AllGathers the full
    normalized matrix over NeuronLink, dma_gathers the positive + 8
    negative candidate rows per row, computes dot products on VectorE
    (bf16 in, f32 accum), top-3 hard negatives via the DVE max op, and
    the per-row logsumexp loss. Host sums the 8192 per-row losses.
  - The compiled program and the jitted SPMD executor are built once and
    cached; steady-state calls only ship ~9.6MB over the axon tunnel.
"""

import numpy as np
from types import SimpleNamespace

B = 8192
D = 512
NCORES = 8
RPC = B // NCORES  # rows per core
P = 128
NTILE = RPC // P  # 8 row-tiles per core
M = 8  # NUM_NEG_CANDIDATES
Q = 1 + M  # gathered rows per target row: positive + 8 negative candidates
KTOP = 256  # precomputed per-row top-K of g_neg (covers max class size + M)
IDXC = (Q * P) // 16  # wrapped idx columns per tile = 72
TEMPERATURE = 0.5
FSCALE = 8.0  # f8 variant: pre-scale so components sit in e4m3 normal range

_CACHE = {}
_VARIANT = "f8"


def _bf16():
    import ml_dtypes

    return np.dtype(ml_dtypes.bfloat16)


def _gumbels():
    """Generate the reference's fixed-key Gumbel matrices on the CPU
    backend (bit-identical threefry bits; log/exp may differ from other
    backends by ULPs, which cannot meaningfully change argmax/top-k of
    continuous random values)."""
    import jax
    import jax.numpy as jnp

    dev = jax.devices("cpu")[0]
    with jax.default_device(dev):
        kp, kn = jax.random.split(jax.random.key(42))
        g_pos = np.asarray(jax.random.gumbel(kp, (B, B), dtype=jnp.float32))
        g_neg = np.asarray(jax.random.gumbel(kn, (B, B), dtype=jnp.float32))
    return g_pos, g_neg


def _init_mining():
    if "g_pos" in _CACHE:
        return
    g_pos, g_neg = _gumbels()
    # Input-independent per-row top-KTOP of g_neg, descending, ties by
    # lower index (the jax.lax.top_k rule).
    part = np.argpartition(-g_neg, KTOP - 1, axis=1)[:, :KTOP]
    part.sort(axis=1)
    vals = np.take_along_axis(g_neg, part, axis=1)
    sel = np.argsort(-vals, axis=1, kind="stable")
    _CACHE["negtop"] = np.take_along_axis(part, sel, axis=1).astype(np.int32)
    _CACHE["g_pos"] = g_pos


def _mine_slow(labels):
    """Exact replication of the reference mining for arbitrary labels
    (only used when a class is so large the top-KTOP table can't cover
    it)."""
    import jax
    import jax.numpy as jnp

    g_pos = _CACHE["g_pos"]
    dev = jax.devices("cpu")[0]
    with jax.default_device(dev):
        _, kn = jax.random.split(jax.random.key(42))
        g_neg = np.asarray(jax.random.gumbel(kn, (B, B), dtype=jnp.float32))
    labels = np.asarray(labels).reshape(-1)
    same = labels[:, None] == labels[None, :]
    neg_inf = np.float32(-np.inf)
    pos_mask = same.copy()
    np.fill_diagonal(pos_mask, False)
    gp = np.where(pos_mask, g_pos, neg_inf)
    pos_j = gp.argmax(axis=1)
    gn = np.where(~same, g_neg, neg_inf)
    KP = 64
    part = np.argpartition(-gn, KP - 1, axis=1)[:, :KP]
    part.sort(axis=1)
    v0 = np.take_along_axis(gn, part, axis=1)
    sel = np.argsort(-v0, axis=1, kind="stable")[:, :M]
    neg_idx = np.take_along_axis(part, sel, axis=1)
    return pos_j, neg_idx


def _mine(labels):
    """Replicates reference mining exactly. Returns pos_j [B], neg_idx [B, M]."""
    _init_mining()
    raw = np.asarray(labels).reshape(-1)
    # mining only depends on the equality classes; remap to 0..K-1 so
    # arbitrary (e.g. negative) label values are safe
    _, labels = np.unique(raw, return_inverse=True)
    counts = np.bincount(labels)
    if counts.max() + M > KTOP:
        return _mine_slow(raw)
    g_pos = _CACHE["g_pos"]
    negtop = _CACHE["negtop"]
    # Negatives: first M different-label entries of the per-row
    # descending top-KTOP — identical to top-8 of the masked row because
    # at most counts.max() same-label entries can precede them.
    lab_t = labels[negtop]
    invalid = lab_t == labels[:, None]
    sel = np.argsort(invalid, axis=1, kind="stable")[:, :M]
    sel.sort(axis=1)  # restore descending-value order among the selected
    neg_idx = np.take_along_axis(negtop, sel, axis=1)
    # Positive: per-class masked argmax (only same-label columns are
    # candidates). First-max tie-break preserved: block columns are in
    # ascending global index order.
    pos_j = np.zeros(B, np.int64)
    order = np.argsort(labels, kind="stable")
    sl = labels[order]
    bounds = np.searchsorted(sl, np.arange(int(sl[-1]) + 2))
    for c in range(len(bounds) - 1):
        rows = order[bounds[c]:bounds[c + 1]]
        n = len(rows)
        if n == 0:
            continue
        if n == 1:
            pos_j[rows[0]] = 0  # all-(-inf) row: argmax is 0
            continue
        block = g_pos[np.ix_(rows, rows)]
        np.fill_diagonal(block, -np.inf)
        pos_j[rows] = rows[np.argmax(block, axis=1)]
    return pos_j, neg_idx


def _wrap_idx(arr):
    """arr: [..., N] index list -> wrapped int16 layout [..., 128, N//16]
    (dma_gather idxs: unwrapped[i] = idxs[i % 16, i // 16], replicated
    across the eight 16-partition blocks)."""
    n = arr.shape[-1]
    s = np.arange(n // 16)
    p = np.arange(P)
    m = s[None, :] * 16 + (p[:, None] % 16)  # [128, n//16]
    return arr[..., m].astype(np.int16)


def _build_program(variant="b"):
    """variant "b": bf16 end-to-end (bf16 AllGather + bf16 gathers).
    variant "e": bf16 on the wire, f32 on the device. variant "f8":
    fp8(e4m3, x8-scaled) on the wire and in HBM, bf16 compute."""
    import concourse.tile as tile
    from concourse import mybir
    from contextlib import ExitStack

    f32 = mybir.dt.float32
    bf16 = mybir.dt.bfloat16
    f8 = mybir.dt.float8e4
    i16 = mybir.dt.int16
    Act = mybir.ActivationFunctionType
    X = mybir.AxisListType.X

    import concourse.bacc as bacc

    nc = bacc.Bacc("TRN2", target_bir_lowering=False, debug=False, num_devices=NCORES)
    # Packed per-core input: cols 0:D = this core's normalized rows,
    # then wrapped positive idx (8 i16) + wrapped negative idxs (64 i16)
    # per row-tile, stored as raw bytes in the blob dtype.
    gdt = {"b": bf16, "e": f32, "f8": f8}[variant]
    wdt = f8 if variant == "f8" else bf16  # wire dtype of the blob
    idxc_w = IDXC * 2 if variant == "f8" else IDXC  # idx cols in blob units
    # logits on device are (SCALE**2)*sims for f8 (features pre-scaled by
    # SCALE to sit in e4m3's normal range); fold 1/SCALE**2 into the
    # logsumexp constants.
    lsc = 2.0 / (FSCALE * FSCALE) if variant == "f8" else 2.0

    blob = nc.declare_dram_parameter("blob", [RPC, D + idxc_w], wdt, isOutput=False)
    lossout = nc.declare_dram_parameter("loss", [NTILE, P], f32, isOutput=True)

    with ExitStack() as ctx:
        tc = ctx.enter_context(tile.TileContext(nc))
        dram = ctx.enter_context(tc.tile_pool(name="dram", bufs=1, space="DRAM"))
        big = ctx.enter_context(tc.tile_pool(name="big", bufs=3))
        mid = ctx.enter_context(tc.tile_pool(name="mid", bufs=3))
        scr = ctx.enter_context(tc.tile_pool(name="scr", bufs=2))
        sml = ctx.enter_context(tc.tile_pool(name="sml", bufs=4))

        agin = dram.tile([RPC, D], gdt)
        fall = dram.tile([B, D], gdt)
        if variant == "e":
            # upconvert the bf16 shard to f32 in DRAM before the collective
            for g in range(NTILE):
                rows = slice(g * P, (g + 1) * P)
                cb = mid.tile([P, D], bf16, tag="cb")
                nc.gpsimd.dma_start(cb[:], blob[rows, 0:D])
                cf = scr.tile([P, D], f32, tag="cf")
                nc.vector.tensor_copy(cf[:], cb[:])
                nc.gpsimd.dma_start(agin[rows, :], cf[:])
        else:
            nc.gpsimd.dma_start(agin[:], blob[:, 0:D])
        nc.gpsimd.collective_compute(
            "AllGather",
            mybir.AluOpType.bypass,
            replica_groups=[list(range(NCORES))],
            ins=[agin[:].opt()],
            outs=[fall[:].opt()],
        )

        for g in range(NTILE):
            rows = slice(g * P, (g + 1) * P)
            git = sml.tile([P, IDXC], i16, tag="git")
            nc.gpsimd.dma_start(git[:], blob[rows, D:D + idxc_w].bitcast(i16))
            xt = mid.tile([P, D], gdt, tag="xt")
            nc.gpsimd.dma_start(xt[:], agin[rows, :])

            pg = mid.tile([P, D], gdt, tag="pg")
            nc.gpsimd.dma_gather(
                pg[:].rearrange("p (q d) -> p q d", q=1),
                fall[:, :],
                git[:, 0:8],
                num_idxs=P,
                num_idxs_reg=P,
                elem_size=D,
            )
            ng = big.tile([P, M * D], gdt, tag="ng")
            nc.gpsimd.dma_gather(
                ng[:].rearrange("p (q d) -> p q d", q=M),
                fall[:, :],
                git[:, 8:IDXC],
                num_idxs=M * P,
                num_idxs_reg=M * P,
                elem_size=D,
            )

            if variant == "f8":
                # upconvert fp8 to bf16 for the DVE multiplies
                xb = mid.tile([P, D], bf16, tag="xb")
                nc.vector.tensor_copy(xb[:], xt[:])
                pb = mid.tile([P, D], bf16, tag="pb")
                nc.vector.tensor_copy(pb[:], pg[:])
                nb = big.tile([P, M * D], bf16, tag="nb")
                nc.vector.tensor_copy(nb[:], ng[:])
                xt, pg, ng = xb, pb, nb
                pdt = bf16
            else:
                pdt = gdt

            # dots col 1 = positive, cols 2..9 = negatives (sims: inputs
            # are unit-normalized, so no norm correction needed)
            prod = scr.tile([P, Q * D], pdt, tag="prod")
            nc.vector.tensor_mul(prod[:, 0:D], xt[:], pg[:])
            for m in range(M):
                nc.vector.tensor_mul(
                    prod[:, (1 + m) * D:(2 + m) * D], xt[:], ng[:, m * D:(m + 1) * D]
                )
            dots = sml.tile([P, 16], f32, tag="dots")
            nc.vector.reduce_sum(
                dots[:, 1:1 + Q],
                prod[:].rearrange("p (m d) -> p m d", m=Q),
                axis=X,
            )

            # top-3 hard negatives (max op returns top-8 sorted desc)
            top8 = sml.tile([P, 8], f32, tag="top8")
            nc.vector.max(top8[:], dots[:, 2:2 + M])

            # logsumexp over logits = sims/T = 2*sims: cols [pos, h1, h2, h3]
            mx = sml.tile([P, 4], f32, tag="mx")
            nc.vector.tensor_max(mx[:, 0:1], dots[:, 1:2], top8[:, 0:1])
            nm2 = sml.tile([P, 4], f32, tag="nm2")
            nc.vector.tensor_scalar_mul(nm2[:, 0:1], mx[:, 0:1], -lsc)
            lg = sml.tile([P, 4], f32, tag="lg")
            nc.vector.tensor_copy(lg[:, 0:1], dots[:, 1:2])
            nc.vector.tensor_copy(lg[:, 1:4], top8[:, 0:3])
            ex = sml.tile([P, 4], f32, tag="ex")
            nc.scalar.activation(ex[:], lg[:], Act.Exp, bias=nm2[:, 0:1], scale=lsc)
            s4 = sml.tile([P, 4], f32, tag="s4")
            nc.vector.reduce_sum(s4[:, 0:1], ex[:], axis=X)
            lns = sml.tile([P, 4], f32, tag="lns")
            nc.scalar.activation(lns[:, 0:1], s4[:, 0:1], Act.Ln)
            # loss = lns + lsc*(mx - pos)
            df = sml.tile([P, 4], f32, tag="df")
            nc.vector.tensor_sub(df[:, 0:1], mx[:, 0:1], dots[:, 1:2])
            lt = sml.tile([P, 4], f32, tag="lt")
            nc.vector.tensor_scalar_mul(lt[:, 0:1], df[:, 0:1], lsc)
            lo = sml.tile([P, 4], f32, tag="lo")
            nc.vector.tensor_add(lo[:, 0:1], lt[:, 0:1], lns[:, 0:1])
            nc.gpsimd.dma_start(lossout[g, :], lo[:, 0:1])

    nc.compile()
    return nc


def _make_executor(nc):
    """Build the jitted SPMD executor once (run_bass_via_pjrt re-jits on
    every call; this is the same lowering with a cached jit)."""
    import jax
    from jax.sharding import Mesh, PartitionSpec
    from jax.experimental.shard_map import shard_map
    from concourse import bass2jax, mybir

    bass2jax.install_neuronx_cc_hook()
    partition_name = nc.partition_id_tensor.name if nc.partition_id_tensor else None
    in_names, out_names, out_avals, zero_shapes = [], [], [], []
    for alloc in nc.m.functions[0].allocations:
        if not isinstance(alloc, mybir.MemoryLocationSet):
            continue
        if alloc.kind not in ("ExternalInput", "ExternalOutput"):
            continue
        name = alloc.memorylocations[0].name
        if alloc.kind == "ExternalInput":
            if name != partition_name:
                in_names.append(name)
        else:
            out_names.append(name)
            shape = tuple(alloc.tensor_shape)
            dtype = mybir.dt.np(alloc.dtype)
            out_avals.append(jax.core.ShapedArray(shape, dtype))
            zero_shapes.append((shape, dtype))
    n_params = len(in_names)
    n_outs = len(out_avals)
    all_in_names = list(in_names) + list(out_names)
    if partition_name is not None:
        all_in_names.append(partition_name)

    def _body(*args):
        operands = list(args)
        if partition_name is not None:
            operands.append(bass2jax.partition_id_tensor())
        outs = bass2jax._bass_exec_p.bind(
            *operands,
            out_avals=tuple(out_avals),
            in_names=tuple(all_in_names),
            out_names=tuple(out_names),
            lowering_input_output_aliases=(),
            sim_require_finite=True,
            sim_require_nnan=True,
            nc=nc,
        )
        return tuple(outs)

    devices = jax.devices()[:NCORES]
    mesh = Mesh(np.asarray(devices), ("core",))
    in_specs = (PartitionSpec("core"),) * (n_params + n_outs)
    out_specs = (PartitionSpec("core"),) * len(out_names)
    # No donation: the kernel writes every loss element, so the pre-zeroed
    # output-backing buffers are never consumed and can live on device
    # across calls.
    sharded = jax.jit(
        shard_map(_body, mesh=mesh, in_specs=in_specs, out_specs=out_specs,
                  check_rep=False),
        keep_unused=True,
    )
    shard = jax.sharding.NamedSharding(mesh, PartitionSpec("core"))
    dev_zeros = [
        jax.device_put(np.zeros((NCORES * s[0], *s[1:]), dt), shard)
        for s, dt in zero_shapes
    ]
    for z in dev_zeros:
        z.block_until_ready()
    return sharded, dev_zeros


def _pack_inputs(features, pos_j, neg_idx, variant="b"):
    import ml_dtypes

    feat = np.asarray(features, dtype=np.float32)
    norms = np.sqrt(np.einsum("ij,ij->i", feat, feat, dtype=np.float32))
    fnorm = feat / np.maximum(norms, np.float32(1e-12))[:, None]
    if variant == "f8":
        wdt = np.dtype(ml_dtypes.float8_e4m3)
        fb = (fnorm * np.float32(FSCALE)).astype(wdt)
    else:
        wdt = np.dtype(ml_dtypes.bfloat16)
        fb = fnorm.astype(wdt)
    idxc_w = (IDXC * 2) // wdt.itemsize  # idx cols in blob dtype units

    # wrapped idx blocks per tile: positive [128, 8], negatives [128, 64]
    pj = pos_j.reshape(NCORES, NTILE, P)
    pw = _wrap_idx(pj)  # [C, T, 128, 8]
    nj = neg_idx.reshape(NCORES, NTILE, P, M).transpose(0, 1, 3, 2)
    nw = _wrap_idx(nj.reshape(NCORES, NTILE, M * P))  # [C, T, 128, 64]
    wrapped = np.concatenate([pw, nw], axis=3)  # [C, T, 128, IDXC]

    blob = np.empty((NCORES, RPC, D + idxc_w), dtype=wdt)
    blob[:, :, :D] = fb.reshape(NCORES, RPC, D)
    blob[:, :, D:] = np.ascontiguousarray(
        wrapped.reshape(NCORES, RPC, IDXC)).view(wdt)
    return blob.reshape(NCORES * RPC, D + idxc_w)


def _run(features, labels, trace=False):
    import time

    variant = _VARIANT
    if "nc" not in _CACHE:
        _CACHE["nc"] = _build_program(variant)
    if "fn" not in _CACHE:
        _CACHE["fn"] = _make_executor(_CACHE["nc"])
    fn, dev_zeros = _CACHE["fn"]

    pos_j, neg_idx = _mine(labels)
    blob = _pack_inputs(features, pos_j, neg_idx, variant)

    t0 = time.time()
    outs = fn(blob, *dev_zeros)
    losses = np.asarray(outs[0], dtype=np.float64).reshape(-1)
    wall_ns = (time.time() - t0) * 1e9
    out = np.float32(losses.sum() / B)
    res = SimpleNamespace(exec_time_ns=None, results=None)
    return out, res, wall_ns


def kernel(features, labels):
    out, _, _ = _run(features, labels)
    return out
